# revision 32
# baseline (speedup 1.0000x reference)
"""Trainium2 Bass kernel for nn_AutoregressiveBisectionInverter.

Inverts y = softplus(s)*x + 0.1*x^3 + tanh(W@x + b) (W strictly lower
triangular) per batch row.  Since W is strictly lower-triangular, the tanh
term at position i depends only on already-solved x_{<i}; each position is
a monotone-cubic scalar root solve.

Strategy (per NeuronCore, batch sharded 1024 -> 8 x 128 rows on the 128
SBUF partitions):
  - Normalize:  x = sqrt(abar)*v with abar = 10*softplus(s)  so the cubic
    becomes p(v) = v^3 + v + dt  (unit coefficients, p' >= 1, |root| <= VM).
  - Per autoregressive step i (serial DVE chain + ScalarE leg):
      tanh + nd = Yt[:,i] - kappa_i*tanh(c_i + b_i)   (two ScalarE activations)
      cnt = #{k: u_k < nd} + seed   (ONE tensor_scalar is_lt+accum over a
                                     host-baked grid u_k = p-poly(v_k);
                                     exact fp32 integer count = ~7 bisections)
      two Newton polish rounds on DVE; round 1 runs in count units with the
      grid pitch h1 folded into the tensor_scalar immediates.
      c_{i+1} = (speculative partial dot) + W'[i+1,i]*v_i   (1-elem fixup)
    The [128,32] partial-dot multiply for row i+1 runs on DVE during step
    i's tanh window (column i of v is still zero there), and its free-axis
    sum runs on the otherwise-idle ScalarE (activation Copy + accum_out).
  - Output x = sqrt(abar)*v (one elementwise mult), DMA out.

Raw bass Blocks are used (TileContext's tail drain trips a sync-wait limit
in this walrus build), with explicit drain() between every same-engine
producer->consumer pair (DVE/ACT pipelines do not interlock RAW hazards).
All input-dependent scalars (kappa_i, b_i via bias tiles, h1, VM, the W'
first-subdiagonal) are baked as instruction immediates at trace time;
broadcasts/grids are precomputed on the host and DMA'd in dependency order
so compute starts after the first three small loads.
"""

import numpy as np

B, D = 1024, 32
NCORES = 8
ROWS = B // NCORES  # 128 rows per core == SBUF partitions
N1 = 128            # bisection-grid points in the fused count op


def _softplus64(x):
    x = x.astype(np.float64)
    return np.log1p(np.exp(-np.abs(x))) + np.maximum(x, 0)


def build(y, W, s, b):
    """Build the SPMD Bass program; returns (nc, in_maps)."""
    from contextlib import ExitStack
    import concourse.bass as bass
    from concourse import mybir

    f32 = mybir.dt.float32
    Alu = mybir.AluOpType

    y = np.ascontiguousarray(y, dtype=np.float32)
    W64 = np.asarray(W, dtype=np.float64)
    s64 = np.asarray(s, dtype=np.float64)
    b64 = np.asarray(b, dtype=np.float64)

    # ---- host precompute ----
    abar = 10.0 * _softplus64(s64)                 # v-linear coefficient
    sqrt_abar = np.sqrt(abar)
    kappa = (10.0 * abar ** -1.5).astype(np.float32)     # per-step immediates
    Yt = (10.0 * y.astype(np.float64) * abar[None, :] ** -1.5).astype(np.float32)
    Wp = np.ascontiguousarray((W64 * sqrt_abar[None, :]).astype(np.float32))
    SA = sqrt_abar.astype(np.float32)[None, :]            # [1, D]
    BT = b64.astype(np.float32)[None, :]                  # [1, D] tanh bias

    dmax = 10.0 * (1.0 + np.abs(y).max(axis=0)) * abar ** -1.5
    VM = float(np.max(np.minimum(np.cbrt(dmax), dmax)) * 1.02 + 1e-3)
    H1 = float(np.float32(2 * VM / (N1 - 1)))
    VM = float(np.float32(VM))
    vk = (-VM + np.arange(N1, dtype=np.float64) * H1)
    UG = ((vk * vk + 1.0) * vk).astype(np.float32)[None, :]   # [1, N1] p-poly
    SEED = float(np.float32(-VM / H1 - 0.5))  # v0 = (count + SEED) * H1

    # ---- build the SPMD Bass program (input-dependent immediates baked) ----
    nc = bass.Bass()
    yt_d = nc.dram_tensor("yt", [ROWS, D], f32, kind="ExternalInput")
    wq_d = nc.dram_tensor("wq", [D, D], f32, kind="ExternalInput")
    ug_d = nc.dram_tensor("ug", [1, N1], f32, kind="ExternalInput")
    sa_d = nc.dram_tensor("sa", [1, D], f32, kind="ExternalInput")
    bt_d = nc.dram_tensor("bt", [1, D], f32, kind="ExternalInput")
    xo_d = nc.dram_tensor("xout", [ROWS, D], f32, kind="ExternalOutput")

    def bcast(ap, parts=ROWS):
        return bass.AP(tensor=ap.tensor, offset=ap.offset,
                       ap=[[0, parts]] + list(ap.ap))

    with ExitStack() as ctx:
        v = ctx.enter_context(nc.sbuf_tensor([ROWS, D], f32))       # v-space solution
        wp = ctx.enter_context(nc.sbuf_tensor([ROWS, D, D], f32))   # W' bcast
        ugt = ctx.enter_context(nc.sbuf_tensor([ROWS, N1], f32))    # poly grid bcast
        sat = ctx.enter_context(nc.sbuf_tensor([ROWS, D], f32))     # sqrt(abar) bcast
        btt = ctx.enter_context(nc.sbuf_tensor([ROWS, D], f32))     # bias bcast
        ytt = ctx.enter_context(nc.sbuf_tensor([ROWS, D], f32))     # Yt shard
        xo = ctx.enter_context(nc.sbuf_tensor([ROWS, D], f32))
        gs = ctx.enter_context(nc.sbuf_tensor([ROWS, N1], f32))     # count scratch
        prod = ctx.enter_context(nc.sbuf_tensor([ROWS, D], f32))
        c = ctx.enter_context(nc.sbuf_tensor([ROWS, 1], f32))
        t = ctx.enter_context(nc.sbuf_tensor([ROWS, 1], f32))
        nd = ctx.enter_context(nc.sbuf_tensor([ROWS, 1], f32))
        cnt = ctx.enter_context(nc.sbuf_tensor([ROWS, 1], f32))
        cpart = ctx.enter_context(nc.sbuf_tensor([ROWS, 1], f32))
        v2 = ctx.enter_context(nc.sbuf_tensor([ROWS, 1], f32))
        v3 = ctx.enter_context(nc.sbuf_tensor([ROWS, 1], f32))
        den = ctx.enter_context(nc.sbuf_tensor([ROWS, 1], f32))
        r = ctx.enter_context(nc.sbuf_tensor([ROWS, 1], f32))
        num = ctx.enter_context(nc.sbuf_tensor([ROWS, 1], f32))
        v1 = ctx.enter_context(nc.sbuf_tensor([ROWS, 1], f32))
        junk = ctx.enter_context(nc.sbuf_tensor([ROWS, D], f32))
        s_dma = ctx.enter_context(nc.semaphore("s_dma"))
        s_dve = ctx.enter_context(nc.semaphore("s_dve"))
        s_act = ctx.enter_context(nc.semaphore("s_act"))
        s_gp = ctx.enter_context(nc.semaphore("s_gp"))
        s_r = ctx.enter_context(nc.semaphore("s_r"))
        block = ctx.enter_context(nc.Block())

        @block.sync
        def _(sync):
            # final store: wait for the vector chain's last inc
            sync.wait_ge(s_dve, D + 1)
            sync.dma_start(out=xo_d[:, :], in_=xo[:, :]).then_inc(s_dma, 16)
            sync.wait_ge(s_dma, 144)

        @block.gpsimd
        def _(gpsimd):
            # Ordered so compute can start after the first small loads land.
            gpsimd.dma_start(out=ytt[:, :], in_=yt_d[:, :]).then_inc(s_dma, 16)
            gpsimd.dma_start(out=btt[:, :], in_=bcast(bt_d[0, :])).then_inc(s_dma, 16)
            gpsimd.dma_start(out=ugt[:, :], in_=bcast(ug_d[0, :])).then_inc(s_dma, 16)
            gpsimd.dma_start(out=wp[:, 0:8, :],
                             in_=bcast(wq_d[0:8, :])).then_inc(s_dma, 16)
            gpsimd.dma_start(out=wp[:, 8:16, :],
                             in_=bcast(wq_d[8:16, :])).then_inc(s_dma, 16)
            gpsimd.dma_start(out=wp[:, 16:24, :],
                             in_=bcast(wq_d[16:24, :])).then_inc(s_dma, 16)
            gpsimd.dma_start(out=wp[:, 24:32, :],
                             in_=bcast(wq_d[24:32, :])).then_inc(s_dma, 16)
            gpsimd.dma_start(out=sat[:, :], in_=bcast(sa_d[0, :])).then_inc(s_dma, 16)


        # NOTE: DVE/ACT pipelines do not interlock same-engine RAW hazards in
        # raw bass -- a dependent back-to-back op reads stale SBUF.  Every
        # producer->consumer edge needs a drain() (pipeline flush) between.
        #
        # The W'@v dot for step i+1 is computed speculatively during step i's
        # tanh window (column i of v is still zero then; the missing
        # W'[i+1,i]*v_i term is added by a 1-element fixup after the solve).
        @block.vector
        def _(vector):
            nc.vector.memset(v[:, :], 0.0)
            nc.vector.memset(c[:, :], 0.0)
            nc.vector.memset(cpart[:, :], 0.0)
            nc.vector.drain().then_inc(s_dve, 1)  # c_0 = 0 ready
            vector.wait_ge(s_dma, 48)  # ytt, bias, grid landed
            for i in range(D):
                if 1 <= i <= D - 2:
                    # speculative partial-dot multiply for row i+1; runs under
                    # tanh_i (column i of v still zero).  The free-axis sum
                    # happens on the idle ScalarE (activation accum).
                    if i == 1:
                        vector.wait_ge(s_dma, 64)
                    elif i + 1 in (8, 16, 24):
                        vector.wait_ge(s_dma, 80 + 16 * ((i + 1) // 8 - 1))
                    nc.vector.tensor_mul(prod[:, :], v[:, :], wp[:, i + 1, :])
                    nc.vector.drain().then_inc(s_gp, 1)
                vector.wait_ge(s_act, i + 1)  # tanh_i + nd affine done
                # count = #{u_k < nd} + SEED  (exact fp32 integer count)
                nc.vector.tensor_scalar(
                    out=gs[:, :], in0=ugt[:, :], scalar1=nd[:, :],
                    scalar2=SEED, op0=Alu.is_lt, op1=Alu.add,
                    accum_out=cnt[:, :])
                nc.vector.drain()
                # Newton round 1 in count units: v0 = cnt*H1
                nc.vector.tensor_mul(v2[:, :], cnt[:, :], cnt[:, :])
                nc.vector.drain()
                nc.vector.tensor_scalar(
                    out=den[:, :], in0=v2[:, :], scalar1=float(3 * H1 * H1),
                    scalar2=1.0, op0=Alu.mult, op1=Alu.add)
                nc.vector.tensor_mul(v3[:, :], v2[:, :], cnt[:, :])
                nc.vector.drain()
                nc.vector.reciprocal(out=r[:, :], in_=den[:, :])
                nc.vector.tensor_scalar(
                    out=num[:, :], in0=v3[:, :], scalar1=float(2 * H1 ** 3),
                    scalar2=nd[:, :], op0=Alu.mult, op1=Alu.add)
                nc.vector.drain()
                nc.vector.tensor_mul(v1[:, :], num[:, :], r[:, :])
                nc.vector.drain()
                # Newton round 2 -> write v[:, i]
                nc.vector.tensor_mul(v2[:, :], v1[:, :], v1[:, :])
                nc.vector.drain()
                nc.vector.tensor_scalar(
                    out=den[:, :], in0=v2[:, :], scalar1=3.0,
                    scalar2=1.0, op0=Alu.mult, op1=Alu.add)
                nc.vector.tensor_mul(v3[:, :], v2[:, :], v1[:, :])
                nc.vector.drain()
                nc.vector.reciprocal(out=r[:, :], in_=den[:, :])
                nc.vector.tensor_scalar(
                    out=num[:, :], in0=v3[:, :], scalar1=2.0,
                    scalar2=nd[:, :], op0=Alu.mult, op1=Alu.add)
                nc.vector.drain()
                nc.vector.tensor_mul(v[:, i:i + 1], num[:, :], r[:, :])
                nc.vector.drain()
                if i <= D - 2:
                    if i >= 1:
                        vector.wait_ge(s_r, i)  # ScalarE partial-dot sum done
                    # c_{i+1} = (partial dot) + W'[i+1,i] * v_i
                    nc.vector.tensor_scalar(
                        out=c[:, :], in0=v[:, i:i + 1],
                        scalar1=float(Wp[i + 1, i]), scalar2=cpart[:, :],
                        op0=Alu.mult, op1=Alu.add)
                    nc.vector.drain().then_inc(s_dve, 1)
            vector.wait_ge(s_dma, 128)  # sqrt(abar) landed
            nc.vector.tensor_mul(xo[:, :], v[:, :], sat[:, :])
            nc.vector.drain().then_inc(s_dve, 1)

        @block.scalar
        def _(scalar):
            scalar.wait_ge(s_dma, 48)  # ytt + bias tiles landed
            for i in range(D):
                scalar.wait_ge(s_dve, i + 1)  # c_i ready
                nc.scalar.activation(
                    out=t[:, :], in_=c[:, :],
                    func=mybir.ActivationFunctionType.Tanh,
                    bias=btt[:, i:i + 1], scale=1.0)
                nc.scalar.drain()
                # nd = -dt = Yt[:,i] - kappa_i * tanh(...)
                nc.scalar.activation(
                    out=nd[:, :], in_=t[:, :],
                    func=mybir.ActivationFunctionType.Identity,
                    bias=ytt[:, i:i + 1], scale=float(-kappa[i]))
                nc.scalar.drain().then_inc(s_act, 1)
                if 1 <= i <= D - 2:
                    # free-axis sum of the speculative partial dot (Copy+accum)
                    scalar.wait_ge(s_gp, i)
                    nc.scalar.activation(
                        out=junk[:, :], in_=prod[:, :],
                        func=mybir.ActivationFunctionType.Copy,
                        bias=0.0, scale=1.0, accum_out=cpart[:, :])
                    nc.scalar.drain().then_inc(s_r, 1)

    in_maps = [
        {"yt": np.ascontiguousarray(Yt[c0 * ROWS:(c0 + 1) * ROWS]),
         "wq": Wp, "ug": UG, "sa": SA, "bt": BT}
        for c0 in range(NCORES)
    ]
    return nc, in_maps


def kernel(y, W, s, b):
    from concourse.bass_utils import run_bass_kernel_spmd

    nc, in_maps = build(y, W, s, b)
    res = run_bass_kernel_spmd(nc, in_maps, list(range(NCORES))).results
    X = np.concatenate([res[c]["xout"] for c in range(NCORES)], axis=0)
    return X.astype(np.float32)


if __name__ == "__main__":
    rng = np.random.default_rng(0)
    y = rng.standard_normal((B, D)).astype(np.float32)
    W = np.tril(rng.standard_normal((D, D)), -1).astype(np.float32) * 0.5
    s = rng.standard_normal(D).astype(np.float32)
    b = rng.standard_normal(D).astype(np.float32)
    X = kernel(y=y, W=W, s=s, b=b)
    print("out", X.shape, X.dtype, X[0, :4])


# revision 33
# speedup vs baseline: 1.2732x; 1.2732x over previous
"""Trainium2 Bass kernel for nn_AutoregressiveBisectionInverter.

Inverts y = softplus(s)*x + 0.1*x^3 + tanh(W@x + b) (W strictly lower
triangular) per batch row.  Since W is strictly lower-triangular, the tanh
term at position i depends only on already-solved x_{<i}; each position is
a monotone-cubic scalar root solve.

Strategy (per NeuronCore, batch sharded 1024 -> 8 x 128 rows on the 128
SBUF partitions):
  - Normalize:  x = sqrt(abar)*v with abar = 10*softplus(s)  so the cubic
    becomes p(v) = v^3 + v + dt  (unit coefficients, p' >= 1, |root| <= VM).
  - Per autoregressive step i (serial DVE chain + ScalarE leg):
      ScalarE: tanh_i = Tanh(W'[i,i-1]*v_{i-1} + cb)  -- the last dot term
               rides tanh's scale, cb = (partial dot + b_i) comes from a
               Copy+accum_out reduce seeded with bias=b_i/D;
               nd = Yt[:,i] - kappa_i*tanh_i  (Identity activation)
      DVE:  cnt = #{k: u_k < nd} + seed   (ONE tensor_scalar is_lt+accum over
              a host-baked grid u_k = p-poly(v_k); exact fp32 count ~ 7
              bisection steps)
            two Newton polish rounds, each as: Horner scan (den), reciprocal,
            Horner scan (num), multiply -- tensor_tensor_scan with a
            stride-0 free-axis broadcast of v evaluates 3v^2+1 and 2v^3+nd
            in one instruction each; round 1 runs in count units with the
            grid pitch h1 folded into the scan initial values.
      The [128,32] partial-dot multiply for row i+1 runs on DVE during step
      i's tanh window (column i of v is still zero there).
  - Output x = sqrt(abar)*v (one elementwise mult), DMA out.

Raw bass Blocks are used (TileContext's tail drain trips a sync-wait limit
in this walrus build), with explicit drain() between every same-engine
producer->consumer pair (DVE/ACT pipelines do not interlock RAW hazards).
All input-dependent scalars are baked as instruction immediates at trace
time; broadcasts/grids are precomputed on the host and DMA'd in dependency
order so compute starts after the first small loads.
"""

import numpy as np

B, D = 1024, 32
NCORES = 8
ROWS = B // NCORES  # 128 rows per core == SBUF partitions
N1 = 128            # bisection-grid points in the fused count op


def _softplus64(x):
    x = x.astype(np.float64)
    return np.log1p(np.exp(-np.abs(x))) + np.maximum(x, 0)


def build(y, W, s, b):
    """Build the SPMD Bass program; returns (nc, in_maps)."""
    from contextlib import ExitStack
    import concourse.bass as bass
    from concourse import mybir

    f32 = mybir.dt.float32
    Alu = mybir.AluOpType
    Act = mybir.ActivationFunctionType

    y = np.ascontiguousarray(np.asarray(y), dtype=np.float32)
    W64 = np.asarray(W, dtype=np.float64)
    s64 = np.asarray(s, dtype=np.float64)
    b64 = np.asarray(b, dtype=np.float64)

    # ---- host precompute ----
    abar = 10.0 * _softplus64(s64)                 # v-linear coefficient
    sqrt_abar = np.sqrt(abar)
    kappa = (10.0 * abar ** -1.5).astype(np.float32)     # per-step immediates
    Yt = (10.0 * y.astype(np.float64) * abar[None, :] ** -1.5).astype(np.float32)
    Wp = np.ascontiguousarray((W64 * sqrt_abar[None, :]).astype(np.float32))
    SA = sqrt_abar.astype(np.float32)[None, :]            # [1, D]
    BT = b64.astype(np.float32)[None, :]                  # [1, D] tanh bias

    dmax = 10.0 * (1.0 + np.abs(y).max(axis=0)) * abar ** -1.5
    VM = float(np.max(np.minimum(np.cbrt(dmax), dmax)) * 1.02 + 1e-3)
    H1 = float(np.float32(2 * VM / (N1 - 1)))
    VM = float(np.float32(VM))
    vk = (-VM + np.arange(N1, dtype=np.float64) * H1)
    UG = ((vk * vk + 1.0) * vk).astype(np.float32)[None, :]   # [1, N1] p-poly
    SEED = float(np.float32(-VM / H1 - 0.5))  # v0 = (count + SEED) * H1

    # ---- build the SPMD Bass program (input-dependent immediates baked) ----
    nc = bass.Bass()
    yt_d = nc.dram_tensor("yt", [ROWS, D], f32, kind="ExternalInput")
    wq_d = nc.dram_tensor("wq", [D, D], f32, kind="ExternalInput")
    ug_d = nc.dram_tensor("ug", [1, N1], f32, kind="ExternalInput")
    sa_d = nc.dram_tensor("sa", [1, D], f32, kind="ExternalInput")
    bt_d = nc.dram_tensor("bt", [1, D], f32, kind="ExternalInput")
    xo_d = nc.dram_tensor("xout", [ROWS, D], f32, kind="ExternalOutput")

    def bcast(ap, parts=ROWS):
        return bass.AP(tensor=ap.tensor, offset=ap.offset,
                       ap=[[0, parts]] + list(ap.ap))

    def frep(ap, k):
        # broadcast a [P,1] AP along the free axis via stride 0
        return bass.AP(tensor=ap.tensor, offset=ap.offset,
                       ap=[list(ap.ap[0]), [0, k]])

    with ExitStack() as ctx:
        v = ctx.enter_context(nc.sbuf_tensor([ROWS, D], f32))       # v-space solution
        wp = ctx.enter_context(nc.sbuf_tensor([ROWS, D, D], f32))   # W' bcast
        ugt = ctx.enter_context(nc.sbuf_tensor([ROWS, N1], f32))    # poly grid bcast
        sat = ctx.enter_context(nc.sbuf_tensor([ROWS, D], f32))     # sqrt(abar) bcast
        btt = ctx.enter_context(nc.sbuf_tensor([ROWS, D], f32))     # bias bcast
        ytt = ctx.enter_context(nc.sbuf_tensor([ROWS, D], f32))     # Yt shard
        xo = ctx.enter_context(nc.sbuf_tensor([ROWS, D], f32))
        gs = ctx.enter_context(nc.sbuf_tensor([ROWS, N1], f32))     # count scratch
        prod = ctx.enter_context(nc.sbuf_tensor([ROWS, D], f32))
        junk = ctx.enter_context(nc.sbuf_tensor([ROWS, D], f32))
        c = ctx.enter_context(nc.sbuf_tensor([ROWS, 1], f32))
        t = ctx.enter_context(nc.sbuf_tensor([ROWS, 1], f32))
        cb = ctx.enter_context(nc.sbuf_tensor([ROWS, 1], f32))      # cpart + b_i
        cnt = ctx.enter_context(nc.sbuf_tensor([ROWS, 1], f32))
        ndt = ctx.enter_context(nc.sbuf_tensor([ROWS, 3], f32))     # [0,0,nd]
        dden = ctx.enter_context(nc.sbuf_tensor([ROWS, 2], f32))    # [0,1]
        scd = ctx.enter_context(nc.sbuf_tensor([ROWS, 2], f32))     # den scan out
        scn = ctx.enter_context(nc.sbuf_tensor([ROWS, 3], f32))     # num scan out
        r = ctx.enter_context(nc.sbuf_tensor([ROWS, 1], f32))
        v1 = ctx.enter_context(nc.sbuf_tensor([ROWS, 1], f32))
        s_dma = ctx.enter_context(nc.semaphore("s_dma"))
        s_dve = ctx.enter_context(nc.semaphore("s_dve"))
        s_act = ctx.enter_context(nc.semaphore("s_act"))
        s_gp = ctx.enter_context(nc.semaphore("s_gp"))
        s_r = ctx.enter_context(nc.semaphore("s_r"))
        s_v = ctx.enter_context(nc.semaphore("s_v"))
        block = ctx.enter_context(nc.Block())

        @block.sync
        def _(sync):
            # final store: wait for the vector chain's last inc
            sync.wait_ge(s_dve, 2)
            sync.dma_start(out=xo_d[:, :], in_=xo[:, :]).then_inc(s_dma, 16)
            sync.wait_ge(s_dma, 144)

        @block.gpsimd
        def _(gpsimd):
            # Ordered so compute can start after the first small loads land.
            gpsimd.dma_start(out=ytt[:, :], in_=yt_d[:, :]).then_inc(s_dma, 16)
            gpsimd.dma_start(out=btt[:, :], in_=bcast(bt_d[0, :])).then_inc(s_dma, 16)
            gpsimd.dma_start(out=ugt[:, :], in_=bcast(ug_d[0, :])).then_inc(s_dma, 16)
            gpsimd.dma_start(out=wp[:, 0:8, :],
                             in_=bcast(wq_d[0:8, :])).then_inc(s_dma, 16)
            gpsimd.dma_start(out=wp[:, 8:16, :],
                             in_=bcast(wq_d[8:16, :])).then_inc(s_dma, 16)
            gpsimd.dma_start(out=wp[:, 16:24, :],
                             in_=bcast(wq_d[16:24, :])).then_inc(s_dma, 16)
            gpsimd.dma_start(out=wp[:, 24:32, :],
                             in_=bcast(wq_d[24:32, :])).then_inc(s_dma, 16)
            gpsimd.dma_start(out=sat[:, :], in_=bcast(sa_d[0, :])).then_inc(s_dma, 16)

        # NOTE: DVE/ACT pipelines do not interlock same-engine RAW hazards in
        # raw bass -- a dependent back-to-back op reads stale SBUF.  Every
        # producer->consumer edge needs a drain() (pipeline flush) between.
        @block.vector
        def _(vector):
            nc.vector.memset(v[:, :], 0.0)
            nc.vector.memset(c[:, :], 0.0)
            nc.vector.memset(ndt[:, :], 0.0)
            nc.vector.memset(dden[:, 0:1], 0.0)
            nc.vector.memset(dden[:, 1:2], 1.0)
            nc.vector.drain().then_inc(s_dve, 1)  # c_0 = 0 / const tiles ready
            vector.wait_ge(s_dma, 48)  # ytt, bias, grid landed
            for i in range(D):
                if 1 <= i <= D - 2:
                    # speculative partial-dot multiply for row i+1; runs under
                    # tanh_i (column i of v is still zero).  The free-axis sum
                    # happens on the otherwise-idle ScalarE.
                    if i == 1:
                        vector.wait_ge(s_dma, 64)
                    elif i + 1 in (8, 16, 24):
                        vector.wait_ge(s_dma, 80 + 16 * ((i + 1) // 8 - 1))
                    if i >= 2:
                        vector.wait_ge(s_r, i - 1)  # ScalarE consumed prod row i
                    nc.vector.tensor_mul(prod[:, :], v[:, :], wp[:, i + 1, :])
                    nc.vector.drain().then_inc(s_gp, 1)
                vector.wait_ge(s_act, i + 1)  # tanh_i + nd affine done
                # count = #{u_k < nd} + SEED  (exact fp32 integer count)
                nc.vector.tensor_scalar(
                    out=gs[:, :], in0=ugt[:, :], scalar1=ndt[:, 2:3],
                    scalar2=SEED, op0=Alu.is_lt, op1=Alu.add,
                    accum_out=cnt[:, :])
                nc.vector.drain()
                # Newton round 1 in count units (v0 = cnt*H1); Horner scans:
                #   den = (3*H1^2*cnt)*cnt + 1 ; num = ((2*H1^3*cnt)*cnt)*cnt + nd
                nc.vector.tensor_tensor_scan(
                    out=scd[:, :], data0=frep(cnt[:, 0:1], 2), data1=dden[:, :],
                    initial=float(3 * H1 * H1), op0=Alu.mult, op1=Alu.add)
                nc.vector.drain()
                nc.vector.reciprocal(out=r[:, :], in_=scd[:, 1:2])
                nc.vector.tensor_tensor_scan(
                    out=scn[:, :], data0=frep(cnt[:, 0:1], 3), data1=ndt[:, :],
                    initial=float(2 * H1 ** 3), op0=Alu.mult, op1=Alu.add)
                nc.vector.drain()
                nc.vector.tensor_mul(v1[:, :], scn[:, 2:3], r[:, :])
                nc.vector.drain()
                # Newton round 2 -> write v[:, i]
                nc.vector.tensor_tensor_scan(
                    out=scd[:, :], data0=frep(v1[:, 0:1], 2), data1=dden[:, :],
                    initial=3.0, op0=Alu.mult, op1=Alu.add)
                nc.vector.drain()
                nc.vector.reciprocal(out=r[:, :], in_=scd[:, 1:2])
                nc.vector.tensor_tensor_scan(
                    out=scn[:, :], data0=frep(v1[:, 0:1], 3), data1=ndt[:, :],
                    initial=2.0, op0=Alu.mult, op1=Alu.add)
                nc.vector.drain()
                nc.vector.tensor_mul(v[:, i:i + 1], scn[:, 2:3], r[:, :])
                if i <= D - 2:
                    nc.vector.drain().then_inc(s_v, 1)
                else:
                    nc.vector.drain()
            vector.wait_ge(s_dma, 128)  # sqrt(abar) landed
            nc.vector.tensor_mul(xo[:, :], v[:, :], sat[:, :])
            nc.vector.drain().then_inc(s_dve, 1)

        @block.scalar
        def _(scalar):
            scalar.wait_ge(s_dma, 48)  # ytt + bias tiles landed
            for i in range(D):
                if i >= 2:
                    # cb = (partial dot of row i) + b_i : Copy+accum with the
                    # per-element bias b_i/D so the sum carries the tanh bias.
                    scalar.wait_ge(s_gp, i - 1)
                    nc.scalar.activation(
                        out=junk[:, :], in_=prod[:, :], func=Act.Copy,
                        bias=float(b64[i] / D), scale=1.0,
                        accum_out=cb[:, :])
                    nc.scalar.drain().then_inc(s_r, 1)
                # tanh_i; the last dot term W'[i,i-1]*v_{i-1} rides the scale
                if i == 0:
                    scalar.wait_ge(s_dve, 1)
                    nc.scalar.activation(
                        out=t[:, :], in_=c[:, :], func=Act.Tanh,
                        bias=btt[:, 0:1], scale=1.0)
                elif i == 1:
                    scalar.wait_ge(s_v, 1)
                    nc.scalar.activation(
                        out=t[:, :], in_=v[:, 0:1], func=Act.Tanh,
                        bias=btt[:, 1:2], scale=float(Wp[1, 0]))
                else:
                    scalar.wait_ge(s_v, i)
                    nc.scalar.activation(
                        out=t[:, :], in_=v[:, i - 1:i], func=Act.Tanh,
                        bias=cb[:, :], scale=float(Wp[i, i - 1]))
                nc.scalar.drain()
                # nd = Yt[:,i] - kappa_i * tanh(...), written into ndt[:,2]
                nc.scalar.activation(
                    out=ndt[:, 2:3], in_=t[:, :], func=Act.Identity,
                    bias=ytt[:, i:i + 1], scale=float(-kappa[i]))
                nc.scalar.drain().then_inc(s_act, 1)

    in_maps = [
        {"yt": np.ascontiguousarray(Yt[c0 * ROWS:(c0 + 1) * ROWS]),
         "wq": Wp, "ug": UG, "sa": SA, "bt": BT}
        for c0 in range(NCORES)
    ]
    return nc, in_maps


def kernel(y, W, s, b):
    from concourse.bass_utils import run_bass_kernel_spmd

    nc, in_maps = build(y, W, s, b)
    res = run_bass_kernel_spmd(nc, in_maps, list(range(NCORES))).results
    X = np.concatenate([res[c]["xout"] for c in range(NCORES)], axis=0)
    return X.astype(np.float32)


if __name__ == "__main__":
    rng = np.random.default_rng(0)
    y = rng.standard_normal((B, D)).astype(np.float32)
    W = np.tril(rng.standard_normal((D, D)), -1).astype(np.float32) * 0.5
    s = rng.standard_normal(D).astype(np.float32)
    b = rng.standard_normal(D).astype(np.float32)
    X = kernel(y=y, W=W, s=s, b=b)
    print("out", X.shape, X.dtype, X[0, :4])


# revision 34
# speedup vs baseline: 1.3097x; 1.0287x over previous
"""Trainium2 Bass kernel for nn_AutoregressiveBisectionInverter.

Inverts y = softplus(s)*x + 0.1*x^3 + tanh(W@x + b) (W strictly lower
triangular) per batch row.  Since W is strictly lower-triangular, the tanh
term at position i depends only on already-solved x_{<i}; each position is
a monotone-cubic scalar root solve.

Strategy (per NeuronCore, batch sharded 1024 -> 8 x 128 rows on the 128
SBUF partitions):
  - Normalize:  x = sqrt(abar)*v with abar = 10*softplus(s)  so the cubic
    becomes p(v) = v^3 + v + dt  (unit coefficients, p' >= 1, |root| <= VM).
  - Per autoregressive step i (serial DVE chain + ScalarE leg):
      ScalarE: tanh_i = Tanh(W'[i,i-1]*v_{i-1} + cb)  -- the last dot term
               rides tanh's scale, cb = (partial dot + b_i) comes from a
               Copy+accum_out reduce seeded with bias=b_i/D;
               nd = Yt[:,i] - kappa_i*tanh_i  (Identity activation)
      DVE:  cnt = #{k: u_k < nd} + seed   (ONE tensor_scalar is_lt+accum over
              a host-baked grid u_k = p-poly(v_k); exact fp32 count ~ 7
              bisection steps)
            two Newton polish rounds, each as: Horner scan (den), reciprocal,
            Horner scan (num), multiply -- tensor_tensor_scan with a
            stride-0 free-axis broadcast of v evaluates 3v^2+1 and 2v^3+nd
            in one instruction each; round 1 runs in count units with the
            grid pitch h1 folded into the scan initial values.
      The [128,32] partial-dot multiply for row i+1 runs on DVE during step
      i's tanh window (column i of v is still zero there).
  - Output x = sqrt(abar)*v (one elementwise mult), DMA out.

Raw bass Blocks are used (TileContext's tail drain trips a sync-wait limit
in this walrus build), with explicit drain() between every same-engine
producer->consumer pair (DVE/ACT pipelines do not interlock RAW hazards).
All input-dependent scalars are baked as instruction immediates at trace
time; broadcasts/grids are precomputed on the host and DMA'd in dependency
order so compute starts after the first small loads.
"""

import numpy as np

B, D = 1024, 32
NCORES = 8
ROWS = B // NCORES  # 128 rows per core == SBUF partitions
N1 = 128            # bisection-grid points in the fused count op


def _softplus64(x):
    x = x.astype(np.float64)
    return np.log1p(np.exp(-np.abs(x))) + np.maximum(x, 0)


def build(y, W, s, b):
    """Build the SPMD Bass program; returns (nc, in_maps)."""
    from contextlib import ExitStack
    import concourse.bass as bass
    from concourse import mybir

    f32 = mybir.dt.float32
    Alu = mybir.AluOpType
    Act = mybir.ActivationFunctionType

    y = np.ascontiguousarray(np.asarray(y), dtype=np.float32)
    W64 = np.asarray(W, dtype=np.float64)
    s64 = np.asarray(s, dtype=np.float64)
    b64 = np.asarray(b, dtype=np.float64)

    # ---- host precompute ----
    abar = 10.0 * _softplus64(s64)                 # v-linear coefficient
    sqrt_abar = np.sqrt(abar)
    kappa = (10.0 * abar ** -1.5).astype(np.float32)     # per-step immediates
    Yt = (10.0 * y.astype(np.float64) * abar[None, :] ** -1.5).astype(np.float32)
    Wp = np.ascontiguousarray((W64 * sqrt_abar[None, :]).astype(np.float32))
    SA = sqrt_abar.astype(np.float32)[None, :]            # [1, D]
    BT = b64.astype(np.float32)[None, :]                  # [1, D] tanh bias

    dmax = 10.0 * (1.0 + np.abs(y).max(axis=0)) * abar ** -1.5
    VM = float(np.max(np.minimum(np.cbrt(dmax), dmax)) * 1.02 + 1e-3)
    H1 = float(np.float32(2 * VM / (N1 - 1)))
    VM = float(np.float32(VM))
    vk = (-VM + np.arange(N1, dtype=np.float64) * H1)
    UG = ((vk * vk + 1.0) * vk).astype(np.float32)[None, :]   # [1, N1] p-poly
    SEED = float(np.float32(-VM / H1 - 0.5))  # v0 = (count + SEED) * H1

    # One header array per core: [ ytt | btt | sat | ugt ] columns, plus a
    # pre-broadcast W' -- exactly two input DMAs (DMA cost here is dominated
    # by the 128 per-partition descriptors, not bytes).
    HW = 3 * D + N1
    WPB = np.ascontiguousarray(np.broadcast_to(Wp[None, :, :], (ROWS, D, D)))

    # ---- build the SPMD Bass program (input-dependent immediates baked) ----
    nc = bass.Bass()
    hd_d = nc.dram_tensor("hdr", [ROWS, HW], f32, kind="ExternalInput")
    wp_d = nc.dram_tensor("wpb", [ROWS, D, D], f32, kind="ExternalInput")
    xo_d = nc.dram_tensor("xout", [ROWS, D], f32, kind="ExternalOutput")

    def frep(ap, k):
        # broadcast a [P,1] AP along the free axis via stride 0
        return bass.AP(tensor=ap.tensor, offset=ap.offset,
                       ap=[list(ap.ap[0]), [0, k]])

    with ExitStack() as ctx:
        v = ctx.enter_context(nc.sbuf_tensor([ROWS, D], f32))       # v-space solution
        wp = ctx.enter_context(nc.sbuf_tensor([ROWS, D, D], f32))   # W' bcast
        hdr = ctx.enter_context(nc.sbuf_tensor([ROWS, HW], f32))
        ytt = hdr[:, 0:D]
        btt = hdr[:, D:2 * D]
        sat = hdr[:, 2 * D:3 * D]
        ugt = hdr[:, 3 * D:3 * D + N1]
        xo = ctx.enter_context(nc.sbuf_tensor([ROWS, D], f32))
        gs = ctx.enter_context(nc.sbuf_tensor([ROWS, N1], f32))     # count scratch
        prod = ctx.enter_context(nc.sbuf_tensor([ROWS, D], f32))
        junk = ctx.enter_context(nc.sbuf_tensor([ROWS, D], f32))
        c = ctx.enter_context(nc.sbuf_tensor([ROWS, 1], f32))
        t = ctx.enter_context(nc.sbuf_tensor([ROWS, 1], f32))
        cb = ctx.enter_context(nc.sbuf_tensor([ROWS, 1], f32))      # cpart + b_i
        cnt = ctx.enter_context(nc.sbuf_tensor([ROWS, 1], f32))
        ndt = ctx.enter_context(nc.sbuf_tensor([ROWS, 3], f32))     # [0,0,nd]
        dden = ctx.enter_context(nc.sbuf_tensor([ROWS, 2], f32))    # [0,1]
        scd = ctx.enter_context(nc.sbuf_tensor([ROWS, 2], f32))     # den scan out
        scn = ctx.enter_context(nc.sbuf_tensor([ROWS, 3], f32))     # num scan out
        r = ctx.enter_context(nc.sbuf_tensor([ROWS, 1], f32))
        v1 = ctx.enter_context(nc.sbuf_tensor([ROWS, 1], f32))
        s_dma = ctx.enter_context(nc.semaphore("s_dma"))
        s_dve = ctx.enter_context(nc.semaphore("s_dve"))
        s_act = ctx.enter_context(nc.semaphore("s_act"))
        s_gp = ctx.enter_context(nc.semaphore("s_gp"))
        s_r = ctx.enter_context(nc.semaphore("s_r"))
        s_v = ctx.enter_context(nc.semaphore("s_v"))
        block = ctx.enter_context(nc.Block())

        @block.sync
        def _(sync):
            # final store: wait for the vector chain's last inc
            sync.wait_ge(s_dve, 2)
            sync.dma_start(out=xo_d[:, :], in_=xo[:, :]).then_inc(s_dma, 16)
            sync.wait_ge(s_dma, 48)

        @block.gpsimd
        def _(gpsimd):
            gpsimd.dma_start(out=hdr[:, :], in_=hd_d[:, :]).then_inc(s_dma, 16)
            gpsimd.dma_start(out=wp[:, :, :], in_=wp_d[:, :, :]).then_inc(s_dma, 16)

        # NOTE: DVE/ACT pipelines do not interlock same-engine RAW hazards in
        # raw bass -- a dependent back-to-back op reads stale SBUF.  Every
        # producer->consumer edge needs a drain() (pipeline flush) between.
        @block.vector
        def _(vector):
            nc.vector.memset(v[:, :], 0.0)
            nc.vector.memset(c[:, :], 0.0)
            nc.vector.memset(ndt[:, :], 0.0)
            nc.vector.memset(dden[:, 0:1], 0.0)
            nc.vector.memset(dden[:, 1:2], 1.0)
            nc.vector.drain().then_inc(s_dve, 1)  # c_0 = 0 / const tiles ready
            vector.wait_ge(s_dma, 16)  # header (ytt/btt/sat/ugt) landed
            for i in range(D):
                if 1 <= i <= D - 2:
                    # speculative partial-dot multiply for row i+1; runs under
                    # tanh_i (column i of v is still zero).  The free-axis sum
                    # happens on the otherwise-idle ScalarE.
                    if i == 1:
                        vector.wait_ge(s_dma, 32)  # W' landed
                    if i >= 2:
                        vector.wait_ge(s_r, i - 1)  # ScalarE consumed prod row i
                    nc.vector.tensor_mul(prod[:, :], v[:, :], wp[:, i + 1, :])
                    nc.vector.drain().then_inc(s_gp, 1)
                vector.wait_ge(s_act, i + 1)  # tanh_i + nd affine done
                # count = #{u_k < nd} + SEED  (exact fp32 integer count)
                nc.vector.tensor_scalar(
                    out=gs[:, :], in0=ugt[:, :], scalar1=ndt[:, 2:3],
                    scalar2=SEED, op0=Alu.is_lt, op1=Alu.add,
                    accum_out=cnt[:, :])
                nc.vector.drain()
                # Newton round 1 in count units (v0 = cnt*H1); Horner scans:
                #   den = (3*H1^2*cnt)*cnt + 1 ; num = ((2*H1^3*cnt)*cnt)*cnt + nd
                nc.vector.tensor_tensor_scan(
                    out=scd[:, :], data0=frep(cnt[:, 0:1], 2), data1=dden[:, :],
                    initial=float(3 * H1 * H1), op0=Alu.mult, op1=Alu.add)
                nc.vector.drain()
                nc.vector.reciprocal(out=r[:, :], in_=scd[:, 1:2])
                nc.vector.tensor_tensor_scan(
                    out=scn[:, :], data0=frep(cnt[:, 0:1], 3), data1=ndt[:, :],
                    initial=float(2 * H1 ** 3), op0=Alu.mult, op1=Alu.add)
                nc.vector.drain()
                nc.vector.tensor_mul(v1[:, :], scn[:, 2:3], r[:, :])
                nc.vector.drain()
                # Newton round 2 -> write v[:, i]
                nc.vector.tensor_tensor_scan(
                    out=scd[:, :], data0=frep(v1[:, 0:1], 2), data1=dden[:, :],
                    initial=3.0, op0=Alu.mult, op1=Alu.add)
                nc.vector.drain()
                nc.vector.reciprocal(out=r[:, :], in_=scd[:, 1:2])
                nc.vector.tensor_tensor_scan(
                    out=scn[:, :], data0=frep(v1[:, 0:1], 3), data1=ndt[:, :],
                    initial=2.0, op0=Alu.mult, op1=Alu.add)
                nc.vector.drain()
                nc.vector.tensor_mul(v[:, i:i + 1], scn[:, 2:3], r[:, :])
                if i <= D - 2:
                    nc.vector.drain().then_inc(s_v, 1)
                else:
                    nc.vector.drain()
            nc.vector.tensor_mul(xo[:, :], v[:, :], sat[:, :])
            nc.vector.drain().then_inc(s_dve, 1)

        @block.scalar
        def _(scalar):
            scalar.wait_ge(s_dma, 16)  # header landed
            for i in range(D):
                if i >= 2:
                    # cb = (partial dot of row i) + b_i : Copy+accum with the
                    # per-element bias b_i/D so the sum carries the tanh bias.
                    scalar.wait_ge(s_gp, i - 1)
                    nc.scalar.activation(
                        out=junk[:, :], in_=prod[:, :], func=Act.Copy,
                        bias=float(b64[i] / D), scale=1.0,
                        accum_out=cb[:, :])
                    nc.scalar.drain().then_inc(s_r, 1)
                # tanh_i; the last dot term W'[i,i-1]*v_{i-1} rides the scale
                if i == 0:
                    scalar.wait_ge(s_dve, 1)
                    nc.scalar.activation(
                        out=t[:, :], in_=c[:, :], func=Act.Tanh,
                        bias=btt[:, 0:1], scale=1.0)
                elif i == 1:
                    scalar.wait_ge(s_v, 1)
                    nc.scalar.activation(
                        out=t[:, :], in_=v[:, 0:1], func=Act.Tanh,
                        bias=btt[:, 1:2], scale=float(Wp[1, 0]))
                else:
                    scalar.wait_ge(s_v, i)
                    nc.scalar.activation(
                        out=t[:, :], in_=v[:, i - 1:i], func=Act.Tanh,
                        bias=cb[:, :], scale=float(Wp[i, i - 1]))
                nc.scalar.drain()
                # nd = Yt[:,i] - kappa_i * tanh(...), written into ndt[:,2]
                nc.scalar.activation(
                    out=ndt[:, 2:3], in_=t[:, :], func=Act.Identity,
                    bias=ytt[:, i:i + 1], scale=float(-kappa[i]))
                nc.scalar.drain().then_inc(s_act, 1)

    in_maps = []
    for c0 in range(NCORES):
        hdr_np = np.concatenate([
            Yt[c0 * ROWS:(c0 + 1) * ROWS],
            np.broadcast_to(BT, (ROWS, D)),
            np.broadcast_to(SA, (ROWS, D)),
            np.broadcast_to(UG, (ROWS, N1)),
        ], axis=1)
        in_maps.append({"hdr": np.ascontiguousarray(hdr_np), "wpb": WPB})
    return nc, in_maps


def kernel(y, W, s, b):
    from concourse.bass_utils import run_bass_kernel_spmd

    nc, in_maps = build(y, W, s, b)
    res = run_bass_kernel_spmd(nc, in_maps, list(range(NCORES))).results
    X = np.concatenate([res[c]["xout"] for c in range(NCORES)], axis=0)
    return X.astype(np.float32)


if __name__ == "__main__":
    rng = np.random.default_rng(0)
    y = rng.standard_normal((B, D)).astype(np.float32)
    W = np.tril(rng.standard_normal((D, D)), -1).astype(np.float32) * 0.5
    s = rng.standard_normal(D).astype(np.float32)
    b = rng.standard_normal(D).astype(np.float32)
    X = kernel(y=y, W=W, s=s, b=b)
    print("out", X.shape, X.dtype, X[0, :4])
